# revision 8
# baseline (speedup 1.0000x reference)
"""Multi-head attention (dense_transformer) on 8 TRN2 NeuronCores.

Sharding: 2-way data parallel over batch x 4-way tensor parallel over heads.
Core c handles batch b=c//4 and heads {4g..4g+3} where g=c%4 (4 heads, 256
channels per core; channels of head h are qw columns {hd*16+h}).

All-fp16 design (validated numerically: rel_err ~3e-3 vs 2e-2 budget):
  phase 1: Q/K projections as 3-term fp16 hi/lo pseudo-fp32 (x and w split
           on host into fp16 pairs); V projection single fp16 term.
           DMA shuffles build per-head stacked score layouts:
             QS[128,h,s] = [Qhi(h); Qlo(h)], KD[128,h,s] = [Khi(h); Khi(h)],
             KL[64,h,s] = Klo(h); V transposed to [k,ch] via DMA XBAR.
  phase 2 (per 512-wide q group): scores in TWO PE passes per q chunk
           (pass A full-128 contraction computes Qh.Kh + Ql.Kh, pass B adds
           Qh.Kl). PSUM drains fuse the causal mask and per-block row max
           via tensor_tensor_reduce; single EXP with sum accumulation;
           attention weights transposed SBUF->SBUF by DMA XBAR (no PE).
           AV packs two heads per PSUM tile via tile_position. One combined
           fp16 AllGather of O^T per group overlaps later groups; the
           out-projection for group g runs during group g+2 (single fp16
           pass, wo rounded to fp16 on host).
"""
import sys

sys.path.insert(0, "/opt/trn_rl_repo")

import numpy as np

import concourse.bass as bass
import concourse.mybir as mybir
import concourse.tile as tile
from concourse import bacc
from concourse.bass_utils import run_bass_kernel_spmd
from concourse.masks import make_causal_mask, make_identity

# ---- problem constants (hardcoded per harness contract) ----
B, S, D, HEADS = 2, 2048, 1024, 16
N_CORES = 8
GROUPS = 4                 # head-groups == cores per batch
HPC = HEADS // GROUPS      # heads per core (4)
HD = D // HEADS            # 64
CPC = HPC * HD             # channels per core (256)
P = 128
NCC = CPC // P             # col chunks per core (2)
DCH = D // P               # contraction chunks (8)
QPG = 4                    # q chunks (128) per 512-wide group

f32 = mybir.dt.float32
f16 = mybir.dt.float16

AX = mybir.AxisListType
EXP = mybir.ActivationFunctionType.Exp
COPY = mybir.ActivationFunctionType.Copy
SUB = mybir.AluOpType.subtract
ADD = mybir.AluOpType.add
MAX = mybir.AluOpType.max
BYP = mybir.AluOpType.bypass
NEG_BIG = -3.0e38

DEFAULT_CFG = dict(s=S, pe_tr=False, split_ag=False, ttr=False,
                   nsplit=True, oshift=2, pshare=True)


def build_nc(s=S, pe_tr=False, split_ag=False, ttr=True, nsplit=True,
             oshift=2, pshare=True, dbg=False):
    assert s % 512 == 0
    NQI = s // P            # q chunks of 128
    NGRP = s // 512         # 512-wide q groups

    nc = bacc.Bacc("TRN2", target_bir_lowering=False, debug=False,
                   num_devices=N_CORES)
    xh_d = nc.dram_tensor("xh", [D, s], f16, kind="ExternalInput").ap()
    xl_d = nc.dram_tensor("xl", [D, s], f16, kind="ExternalInput").ap()
    wdr = {}
    for nm in ("wvh", "wqh", "wql", "wkh", "wkl", "woh"):
        wdr[nm] = nc.dram_tensor(nm, [D, CPC], f16, kind="ExternalInput").ap()
    outT = nc.dram_tensor("outT", [NCC, P, s], f32, kind="ExternalOutput").ap()

    with tile.TileContext(nc) as tc:
        with (
            tc.tile_pool(name="cpool", bufs=1) as cpool,
            tc.tile_pool(name="wop", bufs=1) as wop,
            tc.tile_pool(name="qk", bufs=1) as qkp,
            tc.tile_pool(name="dram", bufs=1, space="DRAM") as dpool,
        ):
            if split_ag:
                ag_in = {(hp, g): dpool.tile([P, 512], f16, tag=f"agi{hp}_{g}",
                                             name=f"agi{hp}_{g}")
                         for hp in range(NCC) for g in range(NGRP)}
                ag_out = {(hp, g): dpool.tile([GROUPS, P, 512], f16,
                                              tag=f"ago{hp}_{g}",
                                              name=f"ago{hp}_{g}")
                          for hp in range(NCC) for g in range(NGRP)}
            else:
                ag_in = {g: dpool.tile([NCC, P, 512], f16, tag=f"agi{g}",
                                       name=f"agi{g}") for g in range(NGRP)}
                ag_out = {g: dpool.tile([GROUPS, NCC, P, 512], f16,
                                        tag=f"ago{g}", name=f"ago{g}")
                          for g in range(NGRP)}

            Wmask = cpool.tile([P, P], f32, tag="Wmask")
            make_causal_mask(nc, Wmask[:], mask_val=-1e10)
            Zrow = cpool.tile([P, 512], f32, tag="Zrow")
            nc.vector.memset(Zrow[:], 0.0)
            if pe_tr:
                ident = cpool.tile([P, P], f16, tag="ident")
                make_identity(nc, ident[:])

            # per-head stacked score layouts + V in [k, ch] layout
            QS = qkp.tile([P, HPC, s], f16, tag="QS")
            KD = qkp.tile([P, HPC, s], f16, tag="KD")
            KL = qkp.tile([64, HPC, s], f16, tag="KL")
            Vsb = qkp.tile([P, NQI, CPC], f16, tag="Vsb")

            woh_sb = wop.tile([P, DCH, CPC], f16, tag="woh", name="woh")

            # ---------------- phase 1: projections ----------------
            with (
                tc.tile_pool(name="wp1", bufs=1) as wp1,
                tc.tile_pool(name="xs", bufs=2) as xs,
                tc.tile_pool(name="stg", bufs=2) as stg,
                tc.tile_pool(name="psp", bufs=1, space="PSUM") as psp,
                tc.tile_pool(name="vtp", bufs=2, space="PSUM") as vtp,
            ):
                wsb = {"woh": woh_sb}
                for nm in ("wvh", "wqh", "wql", "wkh", "wkl"):
                    wsb[nm] = wp1.tile([P, DCH, CPC], f16, tag=nm, name=nm)
                # wvh first: the V matmuls are the first PE work
                for nm in ("wvh", "wqh", "wql", "wkh", "wkl", "woh"):
                    nc.sync.dma_start(wsb[nm][:],
                                      wdr[nm].rearrange("(o p) c -> p o c", p=P))

                for qb in range(NGRP):
                    sl = slice(qb * 512, (qb + 1) * 512)
                    xh_t, xl_t = {}, {}
                    for di in range(DCH):
                        dsl = slice(di * P, (di + 1) * P)
                        xh_t[di] = xs.tile([P, 512], f16, tag=f"xh{di}",
                                           name=f"xh{di}")
                        xl_t[di] = xs.tile([P, 512], f16, tag=f"xl{di}",
                                           name=f"xl{di}")
                        nc.sync.dma_start(xh_t[di][:], xh_d[dsl, sl])
                        nc.sync.dma_start(xl_t[di][:], xl_d[dsl, sl])

                    # V: single fp16 term
                    accV = psp.tile([P, NCC, 512], f32, tag="av", name="av")
                    for cc in range(NCC):
                        csl = slice(cc * P, (cc + 1) * P)
                        for di in range(DCH):
                            nc.tensor.matmul(accV[:, cc, :],
                                             wsb["wvh"][:, di, csl], xh_t[di][:],
                                             start=(di == 0),
                                             stop=(di == DCH - 1))
                    vtb = stg.tile([P, NCC, 512], f16, tag="vtb", name="vtb")
                    for cc in range(NCC):
                        nc.scalar.activation(vtb[:, cc, :], accV[:, cc, :],
                                             COPY)

                    # Q, K: 3-term fp16 pseudo-fp32
                    for nm in ("q", "k"):
                        wh_, wl_ = wsb[f"w{nm}h"], wsb[f"w{nm}l"]
                        acc = psp.tile([P, NCC, 512], f32, tag=f"a{nm}",
                                       name=f"a{nm}")
                        for di in range(DCH):
                            terms = ((wh_, xh_t[di]), (wh_, xl_t[di]),
                                     (wl_, xh_t[di]))
                            for cc in range(NCC):
                                csl = slice(cc * P, (cc + 1) * P)
                                for ti, (wt, xt) in enumerate(terms):
                                    nc.tensor.matmul(
                                        acc[:, cc, :], wt[:, di, csl], xt[:],
                                        start=(di == 0 and ti == 0),
                                        stop=(di == DCH - 1 and ti == 2))
                        hi_t = stg.tile([P, NCC, 512], f16, tag=f"{nm}hi",
                                        name=f"{nm}hi")
                        lo_t = stg.tile([P, NCC, 512], f16, tag=f"{nm}lo",
                                        name=f"{nm}lo")
                        for cc in range(NCC):
                            nc.scalar.activation(hi_t[:, cc, :],
                                                 acc[:, cc, :], COPY)
                            nc.vector.tensor_tensor(lo_t[:, cc, :],
                                                    acc[:, cc, :],
                                                    hi_t[:, cc, :], SUB)
                        # DMA shuffles into per-head stacked layouts
                        for h in range(HPC):
                            cc = h // 2
                            hr = slice((h % 2) * 64, (h % 2) * 64 + 64)
                            if nm == "q":
                                nc.sync.dma_start(QS[0:64, h, sl],
                                                  hi_t[hr, cc, :])
                                nc.sync.dma_start(QS[64:128, h, sl],
                                                  lo_t[hr, cc, :])
                            else:
                                nc.sync.dma_start(KD[0:64, h, sl],
                                                  hi_t[hr, cc, :])
                                nc.sync.dma_start(KD[64:128, h, sl],
                                                  hi_t[hr, cc, :])
                                nc.sync.dma_start(KL[:, h, sl],
                                                  lo_t[hr, cc, :])

                    # V^T -> V transposes
                    for cc in range(NCC):
                        for kl_ in range(4):
                            ki = qb * 4 + kl_
                            src = vtb[:, cc, kl_ * P:(kl_ + 1) * P]
                            dst = Vsb[:, ki, cc * P:(cc + 1) * P]
                            if pe_tr:
                                pt = vtp.tile([P, P], f16, tag="vpt",
                                              name="vpt")
                                nc.tensor.transpose(pt[:], src, ident[:])
                                nc.any.tensor_copy(dst, pt[:])
                            else:
                                nc.sync.dma_start(dst, src, transpose=True)

            # ---------------- phase 2: attention + out-proj ----------------
            with (
                tc.tile_pool(name="scb", bufs=3) as scbp,
                tc.tile_pool(name="atp", bufs=6) as atp,
                tc.tile_pool(name="atT", bufs=3) as atTp,
                tc.tile_pool(name="stat", bufs=8) as stat,
                tc.tile_pool(name="otf", bufs=2) as otfp,
                tc.tile_pool(name="mtp", bufs=1) as mtp,
                tc.tile_pool(name="oop", bufs=2) as oop,
                tc.tile_pool(name="pssc",
                             bufs=4 if (pe_tr or not pshare) else 6,
                             space="PSUM") as pssc,
                tc.tile_pool(name="pacc", bufs=2 if pshare else 1,
                             space="PSUM") as pacc,
                tc.tile_pool(name="pso2", bufs=1, space="PSUM") as pso2,
                tc.tile_pool(name="pspt", bufs=2, space="PSUM") as pspt,
            ):
                def emit_scores(grp, h, ats):
                    for r in range(QPG):
                        qi = grp * QPG + r
                        ktot = (qi + 1) * P
                        nkb = qi // 4 + 1
                        wlast = (qi % 4 + 1) * P
                        qsl = slice(qi * P, (qi + 1) * P)
                        sc_t = []
                        for j in range(nkb):
                            wj = 512 if j < nkb - 1 else wlast
                            st = pssc.tile([P, 512], f32, tag="sc", name="sc")
                            sc_t.append((st, wj))
                        for j, (st, wj) in enumerate(sc_t):
                            ksl = slice(j * 512, j * 512 + wj)
                            nc.tensor.matmul(st[:, :wj], QS[:, h, qsl],
                                             KD[:, h, ksl],
                                             start=True, stop=False)
                        for j, (st, wj) in enumerate(sc_t):
                            ksl = slice(j * 512, j * 512 + wj)
                            nc.tensor.matmul(st[:, :wj], QS[0:64, h, qsl],
                                             KL[:, h, ksl],
                                             start=False, stop=True)
                        # drains with fused per-block row-max; causal mask
                        # fused into the diagonal 128-col drain
                        scb = scbp.tile([P, s], f32, tag="scb", name="scb")
                        negm = stat.tile([P, 1], f32, tag="negm", name="negm")
                        if ttr:
                            mc = stat.tile([P, 8], f32, tag="mc", name="mc")
                            ncol = 0
                            for j, (st, wj) in enumerate(sc_t):
                                if j < nkb - 1:
                                    nc.vector.tensor_tensor_reduce(
                                        scb[:, j * 512:(j + 1) * 512],
                                        st[:], Zrow[:], 1.0, NEG_BIG, ADD, MAX,
                                        mc[:, ncol:ncol + 1])
                                    ncol += 1
                                else:
                                    if wj > P:
                                        nc.vector.tensor_tensor_reduce(
                                            scb[:, j * 512:j * 512 + wj - P],
                                            st[:, :wj - P], Zrow[:, :wj - P],
                                            1.0, NEG_BIG, ADD, MAX,
                                            mc[:, ncol:ncol + 1])
                                        ncol += 1
                                    nc.vector.tensor_tensor_reduce(
                                        scb[:, ktot - P:ktot],
                                        st[:, wj - P:wj], Wmask[:], 1.0,
                                        NEG_BIG, ADD, MAX, mc[:, ncol:ncol + 1])
                                    ncol += 1
                            nc.vector.tensor_reduce(negm[:], mc[:, :ncol],
                                                    axis=AX.X, op=MAX,
                                                    negate=True)
                        else:
                            for j, (st, wj) in enumerate(sc_t):
                                eng = nc.vector if j % 2 == 0 else nc.scalar
                                if j < nkb - 1:
                                    if j % 2 == 0:
                                        nc.vector.tensor_copy(
                                            scb[:, j * 512:(j + 1) * 512],
                                            st[:])
                                    else:
                                        nc.scalar.activation(
                                            scb[:, j * 512:(j + 1) * 512],
                                            st[:], COPY)
                                else:
                                    if wj > P:
                                        nc.scalar.activation(
                                            scb[:, j * 512:j * 512 + wj - P],
                                            st[:, :wj - P], COPY)
                                    nc.vector.tensor_tensor(
                                        scb[:, ktot - P:ktot],
                                        st[:, wj - P:wj], Wmask[:], ADD)
                            nc.vector.tensor_reduce(negm[:], scb[:, :ktot],
                                                    axis=AX.X, op=MAX,
                                                    negate=True)
                        at = atp.tile([P, s], f16, tag="at", name="at")
                        Ssum = stat.tile([P, 1], f32, tag="Ssum", name="Ssum")
                        nc.scalar.activation(at[:, :ktot], scb[:, :ktot], EXP,
                                             bias=negm[:], accum_out=Ssum[:])
                        rec = stat.tile([P, 1], f32, tag="rec", name="rec")
                        nc.vector.reciprocal(rec[:], Ssum[:])
                        if nsplit and ktot >= 1024:
                            half = (ktot // 256) * 128
                            nc.vector.tensor_scalar_mul(
                                at[:, :half], at[:, :half], rec[:])
                            nc.scalar.activation(at[:, half:ktot],
                                                 at[:, half:ktot], COPY,
                                                 scale=rec[:])
                        else:
                            nc.any.tensor_scalar_mul(
                                at[:, :ktot], at[:, :ktot], rec[:])
                        ats[h, r] = at

                def emit_transp(grp, h, ats, atTs):
                    t = atTp.tile([P, NQI, 512], f16, tag="atT", name="atT")
                    atTs[h] = t
                    for dk in range(1, QPG):
                        nc.any.memset(t[:, grp * QPG + dk, 0:dk * P], 0.0)
                    for ki in range(grp * QPG + QPG):
                        rs = max(0, ki - grp * QPG)
                        if pe_tr:
                            pt = pspt.tile([P, 512], f16, tag="pt", name="pt")
                            for r in range(rs, QPG):
                                nc.tensor.transpose(
                                    pt[:, r * P:(r + 1) * P],
                                    ats[h, r][:, ki * P:(ki + 1) * P],
                                    ident[:])
                            nc.any.tensor_copy(t[:, ki, rs * P:512],
                                               pt[:, rs * P:512])
                        else:
                            for r in range(rs, QPG):
                                nc.sync.dma_start(
                                    t[:, ki, r * P:(r + 1) * P],
                                    ats[h, r][:, ki * P:(ki + 1) * P],
                                    transpose=True)

                def emit_av(grp, hp, atTs):
                    nch = grp * QPG + QPG
                    otp = pacc.tile([P, 512], f32, tag="acc", name="otp")
                    for h2 in range(2):
                        vcols = slice(hp * P + h2 * 64, hp * P + (h2 + 1) * 64)
                        for ki in range(nch):
                            nc.tensor.matmul(
                                otp[h2 * 64:(h2 + 1) * 64, :],
                                Vsb[:, ki, vcols], atTs[2 * hp + h2][:, ki, :],
                                start=(ki == 0), stop=(ki == nch - 1),
                                tile_position=(0, h2 * 64))
                    otf = otfp.tile([P, 512], f16, tag="otf", name="otf")
                    nc.vector.tensor_copy(otf[:], otp[:])
                    if split_ag:
                        nc.sync.dma_start(ag_in[hp, grp][:], otf[:])
                        nc.gpsimd.collective_compute(
                            "AllGather", BYP,
                            replica_groups=[[0, 1, 2, 3], [4, 5, 6, 7]],
                            ins=[ag_in[hp, grp][:]], outs=[ag_out[hp, grp][:]])
                    else:
                        nc.sync.dma_start(ag_in[grp][hp], otf[:])

                def emit_gather(grp):
                    if split_ag:
                        return
                    nc.gpsimd.collective_compute(
                        "AllGather", BYP,
                        replica_groups=[[0, 1, 2, 3], [4, 5, 6, 7]],
                        ins=[ag_in[grp][:]], outs=[ag_out[grp][:]])

                def emit_oproj(g):
                    osl = slice(g * 512, (g + 1) * 512)
                    mts = []
                    for mch in range(DCH):
                        g_, cc_ = mch // NCC, mch % NCC
                        mt = mtp.tile([P, 512], f16, tag=f"mt{mch}",
                                      name=f"mt{mch}")
                        if split_ag:
                            nc.sync.dma_start(mt[:], ag_out[cc_, g][g_])
                        else:
                            nc.sync.dma_start(mt[:], ag_out[g][g_, cc_])
                        mts.append(mt)
                    for occ in range(NCC):
                        if pshare:
                            po = pacc.tile([P, 512], f32, tag="acc", name="po")
                        else:
                            po = pso2.tile([P, 512], f32, tag="po", name="po")
                        for mch in range(DCH):
                            nc.tensor.matmul(
                                po[:], woh_sb[:, mch, occ * P:(occ + 1) * P],
                                mts[mch][:], start=(mch == 0),
                                stop=(mch == DCH - 1))
                        oo = oop.tile([P, 512], f32, tag="oo", name="oo")
                        nc.vector.tensor_copy(oo[:], po[:])
                        nc.sync.dma_start(outT[occ, :, osl], oo[:])

                for grp in range(NGRP):
                    ats, atTs = {}, {}
                    emit_scores(grp, 0, ats)
                    emit_scores(grp, 1, ats)
                    emit_transp(grp, 0, ats, atTs)
                    emit_scores(grp, 2, ats)
                    emit_transp(grp, 1, ats, atTs)
                    emit_scores(grp, 3, ats)
                    emit_av(grp, 0, atTs)
                    emit_transp(grp, 2, ats, atTs)
                    if grp >= oshift:
                        emit_oproj(grp - oshift)
                    emit_transp(grp, 3, ats, atTs)
                    emit_av(grp, 1, atTs)
                    emit_gather(grp)
                    if grp == NGRP - 1:
                        for g in range(max(0, NGRP - oshift), NGRP - 1):
                            emit_oproj(g)
                emit_oproj(NGRP - 1)

    nc.compile()
    return nc


_NC_CACHE = {}


def get_nc(**cfg):
    key = tuple(sorted(cfg.items()))
    if key not in _NC_CACHE:
        _NC_CACHE[key] = build_nc(**cfg)
    return _NC_CACHE[key]


def _col_index(g):
    p = np.arange(CPC)
    return (p % HD) * HEADS + (HPC * g + p // HD)


def _ow_row_index():
    r = np.arange(D)
    m, p128 = r // P, r % P
    g_, cc = m // NCC, m % NCC
    p256 = cc * P + p128
    lh, hd = p256 // HD, p256 % HD
    return hd * HEADS + (HPC * g_ + lh)


def _split16(a):
    h = a.astype(np.float16)
    l = (a.astype(np.float32) - h.astype(np.float32)).astype(np.float16)
    return np.ascontiguousarray(h), np.ascontiguousarray(l)


def make_in_maps(x, qw, kw, vw, ow, s=S):
    scale = 1.0 / np.sqrt(np.float32(D))
    qws = (qw * scale).astype(np.float32)
    ow_perm = np.ascontiguousarray(ow[_ow_row_index()])
    xsplit = [_split16(x[b, :s].T) for b in range(B)]
    in_maps = []
    for c in range(N_CORES):
        b, g = c // GROUPS, c % GROUPS
        cols = _col_index(g)
        wqh, wql = _split16(qws[:, cols])
        wkh, wkl = _split16(kw[:, cols])
        in_maps.append({
            "xh": xsplit[b][0], "xl": xsplit[b][1],
            "wqh": wqh, "wql": wql, "wkh": wkh, "wkl": wkl,
            "wvh": np.ascontiguousarray(vw[:, cols].astype(np.float16)),
            "woh": np.ascontiguousarray(
                ow_perm[:, g * CPC:(g + 1) * CPC].astype(np.float16)),
        })
    return in_maps


def assemble_output(results, s=S):
    out = np.empty((B, s, D), dtype=np.float32)
    for c in range(N_CORES):
        b, g = c // GROUPS, c % GROUPS
        oT = results[c]["outT"]  # [NCC, P, s]
        for occ in range(NCC):
            out[b, :, g * CPC + occ * P:(g * CPC + (occ + 1) * P)] = oT[occ].T
    return out


def run_on_hw(x, qw, kw, vw, ow, trace=False, **cfg_over):
    cfg = dict(DEFAULT_CFG)
    cfg.update(cfg_over)
    s = cfg["s"]
    nc = get_nc(**cfg)
    in_maps = make_in_maps(x, qw, kw, vw, ow, s=s)
    res = run_bass_kernel_spmd(nc, in_maps, core_ids=list(range(N_CORES)),
                               trace=trace)
    return assemble_output(res.results, s=s), res


def kernel(x, qw, kw, vw, ow):
    out, _ = run_on_hw(np.asarray(x, dtype=np.float32),
                       np.asarray(qw, dtype=np.float32),
                       np.asarray(kw, dtype=np.float32),
                       np.asarray(vw, dtype=np.float32),
                       np.asarray(ow, dtype=np.float32))
    return out
